# revision 1
# baseline (speedup 1.0000x reference)
"""CapsuleLayer (dynamic routing) Trainium2 Bass kernel.

Problem: u_hat = einsum('bi,crio->bcro', x, W); 3 iterations of dynamic
routing (softmax over capsule dim C, squash over OUT dim) -> v (B, R, OUT).

  B=64, C=32, R=1152, IN=128, OUT=16, ITERS=3.

Strategy (chosen over the batch-parallel hint): shard the ROUTES dim R
across the 8 cores (144 routes each).  Routing is independent per (b, r)
- softmax is over C which stays local - so there are NO collectives, and
each core reads only 1/8 of the 302 MB weight tensor.

Per-core pipeline (fp32 throughout - the routing softmax logits reach
|b|~136 and are extremely sensitive; even 2^-17 relative noise on u_hat
produces ~1e-2 absmax error in v):
  - host pre-permutes the W shard to route-major (r, c, i, o) so DMA tiles
    are (128 rows x 8KB) fully-contiguous loads
  - PE transposes (via identity matmul) rearrange W tiles to put the
    contraction dim IN on partitions
  - fp32 matmuls produce u_hat with partitions = (r-parity, b) = 128 used
  - routing (softmax / weighted sums / squash) on DVE+ACT with 4D access
    patterns; sqrt via exp(0.5*ln) + one Newton step (single ACT table set)
"""

import functools
import os

import numpy as np

B, C, R, IN, OUT = 64, 32, 1152, 128, 16
ITERS = 3
NCORES = 8
RL = R // NCORES            # routes per core = 144
RB = 4                      # routes per hardware tile (x 32 c = 128 partitions)
NT = RL // RB               # 36 tiles per core
G = 9                       # tiles per routing chunk
RC = G * RB                 # routes per chunk = 24
NCH = NT // G               # 6 chunks
RH = RC // 2                # per-lane route slots per chunk = 12
CR = C * RL                 # 4608 (r-major flattened (r, c) pairs)
IO = IN * OUT               # 2048


def _ap(tensor_ap, offset_elems, dims):
    """Manual AP on the same tensor: dims = [[step, count], ...]."""
    import concourse.bass as bass

    return bass.AP(
        tensor=tensor_ap.tensor, offset=tensor_ap.offset + offset_elems, ap=dims
    )


def _bcast(ap, dim_idx, count):
    """Insert a broadcast (stride-0) dim at dim_idx (free dims are 1-based
    after the partition dim)."""
    import concourse.bass as bass

    dims = [list(d) for d in ap.ap]
    dims.insert(dim_idx, [0, count])
    return bass.AP(tensor=ap.tensor, offset=ap.offset, ap=dims)


@functools.lru_cache(maxsize=2)
def _build(debug=False):
    import concourse.bacc as bacc
    import concourse.tile as tile
    from concourse import mybir
    from concourse.masks import make_identity

    f32 = mybir.dt.float32
    AX = mybir.AxisListType
    OP = mybir.AluOpType
    AF = mybir.ActivationFunctionType

    nc = bacc.Bacc(None, target_bir_lowering=False, debug=False)

    w = nc.dram_tensor("w", [CR, IO], f32, kind="ExternalInput")
    x = nc.dram_tensor("x", [B, IN], f32, kind="ExternalInput")
    vout = nc.dram_tensor("v", [B, RL, OUT], f32, kind="ExternalOutput")
    dbg = (
        nc.dram_tensor("dbg", [128, C, RL // 2, OUT], f32, kind="ExternalOutput")
        if debug
        else None
    )

    with tile.TileContext(nc) as tc:
        with (
            tc.tile_pool(name="consts", bufs=1) as consts,
            tc.tile_pool(name="wnat", bufs=3) as wnat_pool,
            tc.tile_pool(name="wt", bufs=3) as wt_pool,
            tc.tile_pool(name="u", bufs=2) as u_pool,
            tc.tile_pool(name="sm", bufs=2) as sm_pool,
            tc.tile_pool(name="tmp", bufs=2) as tmp_pool,
            tc.tile_pool(name="pst", bufs=2, space="PSUM") as psum_t,
            tc.tile_pool(name="psu", bufs=2, space="PSUM") as psum_u,
        ):
            ident = consts.tile([128, 128], f32)
            make_identity(nc, ident)

            # Preload the one ACT table set containing every function we use
            # (Copy/Identity/Square/Ln/Exp) so the auto-inserted per-function
            # loads don't thrash between sets (~2.7us each).
            from concourse.hw_specs import get_activation_tables

            _tabs = list(get_activation_tables(nc.m.arch))
            _set_id = _tabs.index("natural_log_exp_and_others")
            nc.scalar.add_instruction(
                mybir.InstLoadActFuncSet(
                    name=nc.get_next_instruction_name(),
                    ins=[],
                    outs=[],
                    act_func_set_id=_set_id,
                )
            )

            # ---- x -> xT (IN on partitions) ----
            x_sb = consts.tile([B, IN], f32)
            nc.sync.dma_start(out=x_sb[:], in_=x[:])
            xT_ps = psum_u.tile([128, 2, 512], f32, tag="up")
            nc.tensor.transpose(xT_ps[:, 0, 0:B], x_sb[:], ident[0:B, 0:B])
            # x duplicated along M so one matmul yields both partition halves
            # (avoids fp32 stationary loads at array col offset 64, which
            # measured ~1% error - see debug notes)
            xT2 = consts.tile([128, 2, B], f32)
            nc.vector.tensor_copy(xT2[:, 0, :], xT_ps[:, 0, 0:B])
            nc.vector.tensor_copy(xT2[:, 1, :], xT_ps[:, 0, 0:B])

            CHUNKS = [(0, 9), (9, 9), (18, 9), (27, 9)]
            for base, Gq in CHUNKS:
                RHq = 2 * Gq          # per-lane r-slots this chunk
                PH = RHq // 2         # r-slots per sub-chain
                u = u_pool.tile([128, C, RHq, OUT], f32, tag="u", name="u")

                for tau in range(Gq):
                    t = base + tau
                    # ---- load W tile: 128 (r,c) rows x (i,o) 8KB ----
                    wn = wnat_pool.tile([128, IN, OUT], f32)
                    nc.sync.dma_start(
                        out=wn[:],
                        in_=w[128 * t : 128 * (t + 1), :].rearrange(
                            "p (i o) -> p i o", o=OUT
                        ),
                    )
                    # ---- PE transposes: (rc, i)-slices -> (i, rc) per o ----
                    wT = wt_pool.tile([128, 128, OUT], f32)  # (i, rc, o)
                    for half in range(2):
                        tp = psum_t.tile([128, 8, 128], f32, tag="tp")
                        for j in range(8):
                            o = half * 8 + j
                            nc.tensor.matmul(
                                tp[:, j, :],
                                wn[:, :, o],
                                ident[:],
                                is_transpose=True,
                                start=(j % 4 == 0),
                                stop=(j % 4 == 3),
                            )
                        # evac PSUM (i, o8, rc) -> SBUF wT (i, rc, o8)
                        nc.scalar.copy(
                            wT[:, :, half * 8 : half * 8 + 8],
                            tp.rearrange("p o rc -> p rc o"),
                        )
                    # ---- u_hat matmuls: M=128 (x duplicated), one bank per
                    # r_in_tile; all partitions carry the same values ----
                    wT_f = wT.rearrange("p rc o -> p (rc o)")
                    for h in range(2):
                        up = psum_u.tile([128, 2, 512], f32, tag="up")
                        for s in range(2):
                            rit = 2 * h + s
                            nc.tensor.matmul(
                                up[:, s, :],
                                xT2.rearrange("p d b -> p (d b)"),
                                wT_f[:, rit * 512 : (rit + 1) * 512],
                                start=True,
                                stop=True,
                            )
                        # evac: rs = 2*tau + h; rhat=0 rows from slot 0
                        # (r_in_tile even), rhat=1 rows from slot 1 (odd)
                        for s in range(2):
                            nc.scalar.copy(
                                u[64 * s : 64 * s + 64, :, 2 * tau + h, :],
                                up[64 * s : 64 * s + 64, s, :].rearrange(
                                    "p (c o) -> p c o", o=OUT
                                ),
                            )

                if dbg is not None:
                    nc.sync.dma_start(
                        out=dbg[:, :, 2 * base : 2 * base + RHq, :], in_=u[:]
                    )

                # ================= routing on the chunk =================
                # Two independent sub-chains (r-slot halves) so the scheduler
                # can fill one chain's ACT/semaphore gaps with the other's
                # DVE passes.

                for part in range(2):
                    pg = f"{part}"
                    rsl = slice(part * PH, (part + 1) * PH)
                    up_ = u[:, :, rsl, :]  # (128, C, PH, OUT)

                    def stile(shape, tag):
                        return sm_pool.tile(
                            shape, f32, tag=tag + pg, name=tag + pg
                        )

                    def squash(S, extra_scale, rz, tag):
                        """v = squash(s), s = S*extra_scale*rz.  Uses
                        w = (n0^2 + n2) / (2*n0*(1+n2))  (Newton-refined
                        sqrt folded in); one reciprocal total."""
                        sq = stile([128, PH, OUT], "sq")
                        nc.scalar.activation(
                            sq[:], S[:], AF.Square, scale=extra_scale
                        )
                        n2 = stile([128, PH], "n2" + tag)
                        nc.vector.tensor_reduce(n2[:], sq[:], axis=AX.X, op=OP.add)
                        if rz is not None:
                            zq = stile([128, PH], "zq")
                            nc.vector.tensor_mul(zq[:], rz[:], rz[:])
                            nc.vector.tensor_mul(n2[:], n2[:], zq[:])
                        # n0 ~ sqrt(n2) via exp(0.5*ln(n2))
                        n0 = stile([128, PH], "n0")
                        nc.scalar.activation(n0[:], n2[:], AF.Ln)
                        nc.scalar.activation(n0[:], n0[:], AF.Exp, scale=0.5)
                        # den = n0*(1+n2); num = n0^2 + n2; w = num/(2*den)
                        t1 = stile([128, PH], "t1")
                        nc.scalar.add(t1[:], n2[:], 1.0)
                        nc.vector.tensor_mul(t1[:], t1[:], n0[:])
                        nc.vector.reciprocal(t1[:], t1[:])
                        num = stile([128, PH], "num")
                        nc.vector.tensor_mul(num[:], n0[:], n0[:])
                        nc.vector.tensor_add(num[:], num[:], n2[:])
                        wsc = stile([128, PH], "wsc")
                        nc.vector.tensor_mul(wsc[:], num[:], t1[:])
                        if rz is not None:
                            nc.vector.tensor_mul(wsc[:], wsc[:], rz[:])
                        nc.scalar.mul(wsc[:], wsc[:], 0.5 * extra_scale)
                        v = stile([128, PH, OUT], "v" + tag)
                        nc.vector.tensor_mul(v[:], S[:], _bcast(wsc[:], 2, OUT))
                        return v

                    def softmax_e(blog):
                        """unnormalized e = exp(blog - max_c), rz = 1/sum_c e."""
                        m = stile([128, PH], "m")
                        nc.vector.tensor_reduce(
                            m[:],
                            blog.rearrange("p c r -> p r c"),
                            axis=AX.X,
                            op=OP.max,
                        )
                        e = stile([128, C, PH], "e")
                        nc.vector.tensor_sub(e[:], blog[:], _bcast(m[:], 1, C))
                        nc.scalar.activation(e[:], e[:], AF.Exp)
                        rz = stile([128, PH], "z")
                        nc.vector.tensor_reduce(
                            rz[:],
                            e.rearrange("p c r -> p r c"),
                            axis=AX.X,
                            op=OP.add,
                        )
                        nc.vector.reciprocal(rz[:], rz[:])
                        return e, rz

                    def dot_o(vv, out_blog):
                        """out_blog = sum_o u * vv(bcast over c)."""
                        tt = tmp_pool.tile(
                            [128, C, PH, OUT], f32, tag="tt", name="tt"
                        )
                        nc.vector.tensor_mul(
                            tt[:], up_, _bcast(vv[:], 1, C)
                        )
                        nc.vector.tensor_reduce(
                            out_blog[:], tt[:], axis=AX.X, op=OP.add
                        )
                        return out_blog

                    def sum_c(wts, tag):
                        """S = sum_c wts(bcast over o) * u -> (128, PH, OUT)."""
                        S = stile([128, PH, OUT], "S" + tag)
                        tt = tmp_pool.tile(
                            [128, C, PH, OUT], f32, tag="tt", name="tt"
                        )
                        nc.vector.tensor_mul(
                            tt[:], up_, _bcast(wts[:], 3, OUT)
                        )
                        nc.vector.tensor_reduce(
                            S[:],
                            tt.rearrange("p c r o -> p r o c"),
                            axis=AX.X,
                            op=OP.add,
                        )
                        return S

                    # ---- iter 1: coupling uniform = 1/32 ----
                    S1 = stile([128, PH, OUT], "Ssum")
                    nc.vector.tensor_reduce(
                        S1[:],
                        up_.rearrange("p c r o -> p r o c"),
                        axis=AX.X,
                        op=OP.add,
                    )
                    v1 = squash(S1, 1.0 / C, None, "1")

                    # ---- iter 2 ----
                    blog = stile([128, C, PH], "blog")
                    dot_o(v1, blog)  # b2 = u . v1
                    e2, rz2 = softmax_e(blog)
                    S2 = sum_c(e2, "2")
                    v2 = squash(S2, 1.0, rz2, "2")

                    # ---- iter 3 ----
                    g2 = stile([128, C, PH], "g2")
                    dot_o(v2, g2)
                    nc.vector.tensor_add(blog[:], blog[:], g2[:])  # b3
                    e3, rz3 = softmax_e(blog)
                    S3 = sum_c(e3, "3")
                    v3 = squash(S3, 1.0, rz3, "3")

                    # ---- output: v[b, q*RC + 2*(part*PH + rs) + rhat, o] ----
                    for rhat in range(2):
                        nc.sync.dma_start(
                            out=_ap(
                                vout[:],
                                (4 * base + 2 * part * PH + rhat) * OUT,
                                [[RL * OUT, B], [2 * OUT, PH], [1, OUT]],
                            ),
                            in_=v3[64 * rhat : 64 * rhat + 64, :, :],
                        )

    nc.compile()
    return nc


def kernel(x: np.ndarray, route_weights: np.ndarray) -> np.ndarray:
    from concourse.bass_utils import run_bass_kernel_spmd

    debug = bool(int(os.environ.get("CAPS_DEBUG", "0")))
    nc = _build(debug)

    xh = np.ascontiguousarray(
        np.asarray(x, dtype=np.float32).reshape(B, IN)
    )
    W = np.asarray(route_weights, dtype=np.float32)

    in_maps = []
    for k in range(NCORES):
        wk = W[:, k * RL : (k + 1) * RL]          # (C, RL, IN, OUT)
        wk = np.ascontiguousarray(wk.transpose(1, 0, 2, 3)).reshape(CR, IO)
        in_maps.append({"w": wk, "x": xh})

    res = run_bass_kernel_spmd(
        nc,
        in_maps,
        core_ids=list(range(NCORES)),
        trace=bool(int(os.environ.get("CAPS_TRACE", "0"))),
    )
    out = np.concatenate([r["v"] for r in res.results], axis=1)
    if debug:
        kernel.last_dbg = [r["dbg"] for r in res.results]  # type: ignore[attr-defined]
    if bool(int(os.environ.get("CAPS_TRACE", "0"))):
        kernel.last_exec_time_ns = res.exec_time_ns  # type: ignore[attr-defined]
    return out

